# revision 18
# baseline (speedup 1.0000x reference)
"""Trainium2 Bass kernel for nn_BinaryMemoryRNN (scatter_memory).

Computation (reference):
    logits = h_prev @ Mw.T + Mb                 # [B, 28]
    b1/b2  = bits of logits halves (> 0)
    idx1   = clip(sum(b1 * 2^(13-j)), 0, 8191)
    idx2   = clip(sum(b2 * 2^(13-j)), 8192, 16383)
    pre    = x @ Ww.T + h_prev @ Uw.T + mem[idx1] @ Qrw.T + mem[idx2] @ Qlw.T + bias
    out    = sigmoid(layernorm(pre) * gamma + beta)

Strategy: data-parallel over batch across 8 cores (1024 rows each).
  - The four big matmuls run in fp8-e4m3 with MatmulPerfMode.DoubleRow
    (2 K-planes per instruction, 2x bf16 throughput). Weights are scaled
    x256 on host so they sit in e4m3's normal range; the layernorm that
    follows is scale-invariant, so only the bias needs the same x256.
  - logits matmul stays fp32 (index bits are sign-sensitive).
  - memory table replicated in DRAM as e4m3 [16384, 1024]; rows fetched
    with gpsimd.dma_gather (row layout), PE-transposed to [feat, batch]
    in fp8, then fed to DoubleRow matmuls.
  - bias enters the PSUM accumulation as a rank-1 (ones x bias) matmul,
    so the epilogue reads stats straight from PSUM.
  - rstd = 1/sqrt(var+eps) via DVE quake-seed + 2 Newton steps: the ACT
    engine only ever runs Sigmoid, avoiding per-tile activation-table
    reloads.
"""

import sys

sys.path.insert(0, "/opt/trn_rl_repo")

from contextlib import ExitStack

import numpy as np
import ml_dtypes

import concourse.bass as bass
import concourse.tile as tile
from concourse import bacc, mybir, library_config
from concourse.bass_utils import run_bass_kernel_spmd

F32 = mybir.dt.float32
BF16 = mybir.dt.bfloat16
F8E4 = mybir.dt.float8e4
I16 = mybir.dt.int16
I32 = mybir.dt.int32
U16 = mybir.dt.uint16
DR = mybir.MatmulPerfMode.DoubleRow

B, I, H, NB = 8192, 1024, 1024, 14
MEM = 2**NB
NCORES = 8
BL = B // NCORES  # 1024 batch rows per core
KC = H // 128  # 8 contraction chunks
KP = KC // 2  # 4 DoubleRow k-pairs
MT = BL // 128  # 8 output row-tiles per core
EPS = 1e-5
WSCALE = 256.0
EPS_SC = EPS * WSCALE * WSCALE  # eps for the x256-scaled pre-activation
RSQRT_MAGIC = 0x5EF759DF  # 0x5f3759df - 0x00400000: seed for rsqrt(2*vh)

# const_t packed layout (f32 columns)
C_MW = 0  # [128, 224] Mw in [p, k*28+j]
C_CLIP = 224  # [2, 2] idx clip bounds
C_NEGMB = 226  # [28, 1] -Mb
C_PW = 227  # [28, 1] powers of two as bf16 pair-packed in f32
C_IDENT = 228  # [128, 32] 128x128 fp8 identity (bitcast)
C_MAGIC = 260  # [128, 1] rsqrt seed magic (int32 bits)
NCONST = 261

_CACHE = {}


def _build(trivial_gb: bool):
    """Trace the Bass/Tile module (shared by all 8 cores, SPMD)."""
    nc = bacc.Bacc(
        "TRN2", target_bir_lowering=False, debug=False, enable_asserts=True
    )

    x8_t = nc.dram_tensor("x8_t", [128, KC, BL], F8E4, kind="ExternalInput").ap()
    h8_t = nc.dram_tensor("h8_t", [128, KC, BL], F8E4, kind="ExternalInput").ap()
    h32_t = nc.dram_tensor("h32_t", [128, KC, BL], F32, kind="ExternalInput").ap()
    # weights, [src, feat_in(part), feat_in(chunk), feat_out]; src order W,U,Qr,Ql
    w8_t = nc.dram_tensor("w8_t", [4, 128, KC, H], F8E4, kind="ExternalInput").ap()
    bias8_t = nc.dram_tensor("bias8_t", [1, H], F8E4, kind="ExternalInput").ap()
    const_t = nc.dram_tensor("const_t", [128, NCONST], F32, kind="ExternalInput").ap()
    mem_t = nc.dram_tensor("mem_t", [MEM, H], F8E4, kind="ExternalInput").ap()
    if not trivial_gb:
        gam_t = nc.dram_tensor("gam_t", [128, H], F32, kind="ExternalInput").ap()
        bet_t = nc.dram_tensor("bet_t", [128, H], F32, kind="ExternalInput").ap()
    out_t = nc.dram_tensor("out_t", [BL, H], BF16, kind="ExternalOutput").ap()

    with tile.TileContext(nc) as tc:
        with ExitStack() as ctx:
            # ---------------- pools ----------------
            cpool = ctx.enter_context(tc.tile_pool(name="consts", bufs=1))
            apool = ctx.enter_context(tc.tile_pool(name="acts", bufs=1))
            hpool = ctx.enter_context(tc.tile_pool(name="h32mid", bufs=3))
            gpool = ctx.enter_context(tc.tile_pool(name="gathered", bufs=1))
            spool = ctx.enter_context(tc.tile_pool(name="small", bufs=2))
            epool = ctx.enter_context(tc.tile_pool(name="epilogue", bufs=2))
            pp_main = ctx.enter_context(
                tc.tile_pool(name="psum_main", bufs=2, space="PSUM")
            )
            # logits / idx / PE-transpose outputs share two 2-bank slots
            pp_small = ctx.enter_context(
                tc.tile_pool(name="psum_small", bufs=2, space="PSUM")
            )

            # gpsimd ucode library containing DMAGatherAnt; load it up front
            # so the Q7 IRAM reload overlaps the initial DMAs.
            nc.gpsimd.load_library(library_config.attnmlp)

            # ---------------- input loads ----------------
            # critical path first: packed consts + h fp32 for the index pipeline
            const_sb = cpool.tile([128, NCONST], F32, tag="const")
            nc.sync.dma_start(const_sb[:], const_t[:])
            mw_sb = const_sb[:, C_MW : C_MW + 224].rearrange(
                "p (k j) -> p k j", j=2 * NB
            )
            clip_sb = const_sb[0:2, C_CLIP : C_CLIP + 2]
            negmb_sb = const_sb[0 : 2 * NB, C_NEGMB : C_NEGMB + 1]
            pw_sb = const_sb[0 : 2 * NB, C_PW : C_PW + 1].bitcast(BF16)
            ident_sb = const_sb[:, C_IDENT : C_IDENT + 32].bitcast(F8E4)
            magic_sb = const_sb[:, C_MAGIC : C_MAGIC + 1].bitcast(I32)

            # warm the Sigmoid activation table while DMAs run (the only
            # ACT function in this kernel -> one table load total)
            warm = cpool.tile([128, 1], F32, tag="warm")
            nc.vector.memset(warm[:], 0.0)
            nc.scalar.activation(
                warm[:], warm[:], mybir.ActivationFunctionType.Sigmoid
            )

            ones_sb = cpool.tile([1, 128], F8E4, tag="ones")
            nc.vector.memset(ones_sb[:], 1.0)
            bias8_sb = cpool.tile([1, H], F8E4, tag="bias8")
            nc.sync.dma_start(bias8_sb[:], bias8_t[:])

            # h32 split 1/2/2/2/1 chunks: the logits matmul starts after the
            # first 512KB
            h32_k0 = spool.tile([128, 1, BL], F32, tag="h32k0")
            nc.sync.dma_start(h32_k0[:], h32_t[:, 0:1, :])
            h32_mid = []
            for piece in range(3):
                hp = hpool.tile([128, 2, BL], F32, tag="slab")
                nc.sync.dma_start(
                    hp[:], h32_t[:, 1 + 2 * piece : 3 + 2 * piece, :]
                )
                h32_mid.append(hp)
            h32_k7 = spool.tile([128, 1, BL], F32, tag="h32k7")
            nc.sync.dma_start(h32_k7[:], h32_t[:, KC - 1 : KC, :])

            def h32_chunk(k):
                if k == 0:
                    return h32_k0[:, 0, :]
                if k == KC - 1:
                    return h32_k7[:, 0, :]
                return h32_mid[(k - 1) // 2][:, (k - 1) % 2, :]

            x8_sb = apool.tile([128, KC, BL], F8E4, tag="x8")
            nc.sync.dma_start(x8_sb[:], x8_t[:])
            h8_sb = apool.tile([128, KC, BL], F8E4, tag="h8")
            nc.sync.dma_start(h8_sb[:], h8_t[:])
            w_sb = []
            for s in range(4):
                w = cpool.tile([128, KC, H], F8E4, tag=f"w{s}")
                nc.sync.dma_start(w[:], w8_t[s])
                w_sb.append(w)
            if not trivial_gb:
                gam_sb = cpool.tile([128, H], F32, tag="gam")
                nc.sync.dma_start(gam_sb[:], gam_t[:])
                bet_sb = cpool.tile([128, H], F32, tag="bet")
                nc.sync.dma_start(bet_sb[:], bet_t[:])
                zero_sb = cpool.tile([128, 1], F32, tag="zero")
                nc.vector.memset(zero_sb[:], 0.0)

            # ---------------- index pipeline ----------------
            # logits.T [28, BL] fp32, accumulated over KC chunks; k-outer so
            # the first half of h32 is enough to start
            logit_ps = pp_small.tile([2 * NB, BL], F32, tag="sm")
            for k in range(KC):
                hk = h32_chunk(k)
                for n in range(BL // 512):
                    nc.tensor.matmul(
                        logit_ps[:, n * 512 : (n + 1) * 512],
                        mw_sb[:, k, :],
                        hk[:, n * 512 : (n + 1) * 512],
                        start=(k == 0),
                        stop=(k == KC - 1),
                    )
            # bits = (h@Mw.T + Mb > 0)  <=>  (h@Mw.T > -Mb), as 1.0/0.0
            bits_sb = spool.tile([2 * NB, BL], BF16, tag="bits")
            nc.vector.tensor_scalar(
                bits_sb[:], logit_ps[:], negmb_sb[:, 0:1], None,
                mybir.AluOpType.is_gt,
            )
            # raw indices via tiny matmul with powers of two: [2, BL]
            idx_ps = pp_small.tile([2, BL], F32, tag="sm")
            for n in range(BL // 512):
                nc.tensor.matmul(
                    idx_ps[:, n * 512 : (n + 1) * 512],
                    pw_sb,
                    bits_sb[:, n * 512 : (n + 1) * 512],
                    start=True,
                    stop=True,
                )
            # clip + cast to int16; per-partition clip bounds:
            # row0 -> [0, 8191], row1 -> [8192, 16383]
            idx16 = spool.tile([2, BL], I16, tag="idx16")
            nc.vector.tensor_scalar(
                idx16[:], idx_ps[:], clip_sb[:, 0:1], clip_sb[:, 1:2],
                mybir.AluOpType.max, mybir.AluOpType.min,
            )

            # Wrap each index row into the [16, BL/16] layout dma_gather wants,
            # replicated to every 16-partition group (the Q7 ucode cores each
            # read their own group). Stage S[i, 32j+q'] = idx[(32j+i)*16+q'%16]
            # (16 columns duplicated within each 32-block), then four DVE
            # 32x32 block-transposes to partition bases 0/32/64/96.
            idxw_r = []
            for r in range(2):
                # issue on ACT's HWDGE FIFO so this tiny latency-critical
                # transfer doesn't queue behind the big input loads on SP's
                stg = spool.tile([32, 64], I16, tag="stage")
                stg_j = stg[0:32, :].rearrange("p (j hq) -> p j hq", j=2)
                with nc.allow_non_contiguous_dma(reason="tiny idx wrap staging"):
                    for j in range(2):
                        nc.scalar.dma_start(
                            stg[0:32, 32 * j : 32 * j + 16],
                            idx16[r : r + 1, j * 512 : (j + 1) * 512].rearrange(
                                "p (a b) -> p a b", b=16
                            ),
                        )
                nc.vector.tensor_copy(stg_j[:, :, 16:32], stg_j[:, :, 0:16])
                idxw = spool.tile([128, 64], I16, tag="idxw")
                for g in range(4):
                    nc.vector.transpose(idxw[32 * g : 32 * (g + 1), :], stg[:])
                idxw_r.append(idxw)

            # gathers split in batch halves, interleaved r0/r1, so blocks
            # c=0-3 of BOTH tensors arrive after the first two half-gathers.
            # g2[r][hf][p, c, :] = mem[idx_{(4*hf+c)*128+p}, :]  (fp8 rows)
            HB = BL // 2
            g2_tiles = [[None, None], [None, None]]
            for hf in range(2):
                for r in range(2):
                    g2 = gpool.tile([128, HB // 128, H], F8E4, tag=f"g2_{r}{hf}")
                    nc.gpsimd.dma_gather(
                        out_ap=g2[:],
                        in_ap=mem_t[:],
                        idxs_ap=idxw_r[r][:, hf * 32 : (hf + 1) * 32],
                        num_idxs=HB,
                        num_idxs_reg=HB,
                        elem_size=H,
                        transpose=False,
                    )
                    g2_tiles[r][hf] = g2

            # ---------------- main matmuls + epilogue ----------------
            srcs_xh = [(x8_sb, 0), (h8_sb, 1)]
            ps_tiles = {}
            mem_sb = [[None] * MT, [None] * MT]

            def emit_xh(m):
                ps = pp_main.tile([128, H], F32, tag="acc")
                ps_tiles[m] = ps
                ms = slice(m * 128, (m + 1) * 128)
                # rank-1 bias matmul opens the accumulation group
                for n in range(H // 512):
                    nc.tensor.matmul(
                        ps[:, n * 512 : (n + 1) * 512],
                        ones_sb[:],
                        bias8_sb[:, n * 512 : (n + 1) * 512],
                        start=True,
                        stop=False,
                    )
                for si, (act, wi) in enumerate(srcs_xh):
                    for kp in range(KP):
                        lhs = act[:, 2 * kp : 2 * kp + 2, ms]
                        for n in range(H // 512):
                            nc.tensor.matmul(
                                ps[:, n * 512 : (n + 1) * 512],
                                lhs,
                                w_sb[wi][:, 2 * kp : 2 * kp + 2,
                                         n * 512 : (n + 1) * 512],
                                start=False,
                                stop=False,
                                perf_mode=DR,
                            )

            def emit_transpose(c):
                # PE-transpose gathered fp8 rows of batch-block c into
                # [feat, batch] fp8 tiles. The fp8 transpose datapath works
                # in 16-bit lanes, so the output lands at element step 2;
                # the DVE copy compacts it.
                for r in range(2):
                    g2 = g2_tiles[r][c // 4]
                    cc = c % 4
                    mt = gpool.tile([128, KC, 128], F8E4, tag=f"mem{r}_{c}")
                    for k in range(KC):
                        tp = pp_small.tile([128, 256], F8E4, tag="sm")
                        tp_s = tp.rearrange("p (b j) -> p b j", j=2)[:, :, 0]
                        nc.tensor.transpose(
                            tp_s, g2[:, cc, k * 128 : (k + 1) * 128],
                            ident_sb[:],
                        )
                        nc.vector.tensor_copy(mt[:, k, :], tp_s)
                    mem_sb[r][c] = mt

            def emit_mem_epilogue(m):
                ps = ps_tiles.pop(m)
                ms = slice(m * 128, (m + 1) * 128)
                for si in range(2):
                    mt = mem_sb[si][m]  # [128, KC, 128] block for this m
                    for kp in range(KP):
                        lhs = mt[:, 2 * kp : 2 * kp + 2, :]
                        for n in range(H // 512):
                            nc.tensor.matmul(
                                ps[:, n * 512 : (n + 1) * 512],
                                lhs,
                                w_sb[2 + si][:, 2 * kp : 2 * kp + 2,
                                             n * 512 : (n + 1) * 512],
                                start=False,
                                stop=(si == 1 and kp == KP - 1),
                                perf_mode=DR,
                            )

                # layernorm stats straight from PSUM (bias already inside)
                st6 = epool.tile([128, 2, 6], F32, tag="st6")
                for a in range(2):
                    nc.vector.bn_stats(st6[:, a, :], ps[:, a * 512 : (a + 1) * 512])
                mv = epool.tile([128, 2], F32, tag="mv")
                nc.vector.bn_aggr(mv[:], st6.rearrange("p a b -> p (a b)"))
                # rstd = 1/sqrt(var+eps) entirely on DVE:
                # quake seed from vh=(var+eps)/2 bits, then 2 Newton steps
                # y <- y*(1.5 - vh*y^2).
                st = epool.tile([128, 4], F32, tag="rs")
                vh = st[:, 0:1]
                y = st[:, 1:2]
                a_ = st[:, 2:3]
                nmu = st[:, 3:4]
                nc.vector.tensor_scalar(
                    vh, mv[:, 1:2], 0.5, EPS_SC * 0.5,
                    mybir.AluOpType.mult, mybir.AluOpType.add,
                )
                nc.vector.tensor_scalar(
                    a_.bitcast(I32), vh.bitcast(I32), 1, None,
                    mybir.AluOpType.logical_shift_right,
                )
                nc.vector.tensor_tensor(
                    y.bitcast(I32), magic_sb[:], a_.bitcast(I32),
                    mybir.AluOpType.subtract,
                )
                for _ in range(2):
                    nc.vector.tensor_tensor(a_, y, y, mybir.AluOpType.mult)
                    nc.vector.tensor_tensor(a_, a_, vh, mybir.AluOpType.mult)
                    nc.vector.tensor_scalar(
                        a_, a_, 1.5, -1.0,
                        mybir.AluOpType.subtract, mybir.AluOpType.mult,
                    )
                    nc.vector.tensor_tensor(y, y, a_, mybir.AluOpType.mult)
                nc.vector.tensor_scalar(
                    nmu, mv[:, 0:1], y, -1.0,
                    mybir.AluOpType.mult, mybir.AluOpType.mult,
                )
                o = epool.tile([128, H], BF16, tag="o")
                if trivial_gb:
                    # out = sigmoid((t - mu) * rstd), read from PSUM
                    nc.scalar.activation(
                        o[:], ps[:], mybir.ActivationFunctionType.Sigmoid,
                        bias=nmu, scale=y,
                    )
                else:
                    xh = epool.tile([128, H], F32, tag="xh")
                    nc.scalar.activation(
                        xh[:], ps[:], mybir.ActivationFunctionType.Identity,
                        bias=nmu, scale=y,
                    )
                    nc.vector.tensor_tensor(
                        xh[:], xh[:], gam_sb[:], mybir.AluOpType.mult
                    )
                    nc.vector.tensor_tensor(
                        xh[:], xh[:], bet_sb[:], mybir.AluOpType.add
                    )
                    nc.scalar.activation(
                        o[:], xh[:], mybir.ActivationFunctionType.Sigmoid,
                        bias=zero_sb[:, 0:1],
                    )
                nc.sync.dma_start(out_t[ms, :], o[:])

            emit_xh(0)
            emit_xh(1)
            for c in range(4):
                emit_transpose(c)
            emit_mem_epilogue(0)
            emit_xh(2)
            emit_mem_epilogue(1)
            emit_xh(3)
            for c in range(4, 8):
                emit_transpose(c)
            emit_mem_epilogue(2)
            for m in range(4, MT):
                emit_xh(m)
                emit_mem_epilogue(m - 1)
            emit_mem_epilogue(MT - 1)

    nc.compile()  # bacc register allocation / DCE
    return nc


def _to_kxp(a, dtype):
    """[batch, feat] -> [128, KC, batch] with feat = k*128 + p."""
    t = np.ascontiguousarray(a.T.reshape(KC, 128, -1).transpose(1, 0, 2))
    return t.astype(dtype)


def prep(inputs):
    """Host-side shard/layout prep. Returns (in_maps, trivial_gb)."""
    x = np.asarray(inputs["x"], np.float32)
    h = np.asarray(inputs["h_prev"], np.float32)
    memory = np.asarray(inputs["memory"], np.float32)
    gamma = np.asarray(inputs["gamma"], np.float32)
    beta = np.asarray(inputs["beta"], np.float32)
    trivial_gb = bool(np.all(gamma == 1.0) and np.all(beta == 0.0))

    bf = ml_dtypes.bfloat16
    e4 = ml_dtypes.float8_e4m3
    # W is [out, in]; the kernel wants w[p, k, n] = W[n, k*128+p], which is
    # exactly _to_kxp applied to W with (out, in) in the (batch, feat) slots.
    w_cat = np.stack(
        [
            _to_kxp(np.asarray(inputs[n], np.float32) * WSCALE, e4)
            for n in ("Ww", "Uw", "Qrw", "Qlw")
        ]
    )
    mw = _to_kxp(np.asarray(inputs["Mw"], np.float32), np.float32)  # [128, KC, 28]

    pw = np.zeros((2 * NB, 2), np.float32)
    pw[:NB, 0] = 2.0 ** np.arange(NB - 1, -1, -1)
    pw[NB:, 1] = 2.0 ** np.arange(NB - 1, -1, -1)
    clip = np.array(
        [[0.0, MEM // 2 - 1], [MEM // 2, MEM - 1]], np.float32
    )  # [row, (lo, hi)]

    mem8 = memory.astype(e4)
    ident8 = np.eye(128, dtype=np.float32).astype(e4)
    bias8 = (
        (
            np.asarray(inputs["Wb"], np.float32)
            + np.asarray(inputs["Ub"], np.float32)
            + np.asarray(inputs["Qrb"], np.float32)
            + np.asarray(inputs["Qlb"], np.float32)
        )
        * WSCALE
    ).astype(e4).reshape(1, H)

    # pack the small constants into one [128, NCONST] f32 buffer
    const = np.zeros((128, NCONST), np.float32)
    const[:, C_MW : C_MW + 224] = mw.reshape(128, 224)
    const[:2, C_CLIP : C_CLIP + 2] = clip
    const[: 2 * NB, C_NEGMB : C_NEGMB + 1] = -np.asarray(
        inputs["Mb"], np.float32
    ).reshape(2 * NB, 1)
    const[: 2 * NB, C_PW : C_PW + 1] = pw.astype(bf).view(np.float32)
    const[:, C_IDENT : C_IDENT + 32] = ident8.view(np.float32)
    const[:, C_MAGIC : C_MAGIC + 1] = (
        np.full((128, 1), RSQRT_MAGIC, np.int32).view(np.float32)
    )

    common = dict(w8_t=w_cat, bias8_t=bias8, const_t=const, mem_t=mem8)
    if not trivial_gb:
        common["gam_t"] = np.ascontiguousarray(np.broadcast_to(gamma, (128, H)))
        common["bet_t"] = np.ascontiguousarray(np.broadcast_to(beta, (128, H)))

    in_maps = []
    for c in range(NCORES):
        xs = x[c * BL : (c + 1) * BL]
        hs = h[c * BL : (c + 1) * BL]
        in_maps.append(
            dict(
                x8_t=_to_kxp(xs, e4),
                h8_t=_to_kxp(hs, e4),
                h32_t=_to_kxp(hs, np.float32),
                **common,
            )
        )
    return in_maps, trivial_gb


def get_nc(trivial_gb):
    key = ("nc", trivial_gb)
    if key not in _CACHE:
        _CACHE[key] = _build(trivial_gb)
    return _CACHE[key]


def run(inputs, trace=False, **kw):
    in_maps, trivial_gb = prep(inputs)
    nc = get_nc(trivial_gb)
    res = run_bass_kernel_spmd(
        nc, in_maps, core_ids=list(range(NCORES)), trace=trace, **kw
    )
    out = np.concatenate([res.results[c]["out_t"] for c in range(NCORES)], axis=0)
    return out.astype(np.float32), res


def kernel(**inputs):
    return run(inputs)[0]
